# revision 8
# baseline (speedup 1.0000x reference)
"""Exponential smoother: out[b,n] = sum_t w[t] * x[b,t,n], with
w = normalized exp(-t/tau) decay weights (tau=20).

Strategy (8 NeuronCores, pure data parallel over B=64):
  - each core handles 8 batches of x[8, 1000, 4096] f32.
  - w decays so fast that t >= 384 contributes < 2.2e-9 absolute
    (~5e-9 relative) -- below half-ulp of the fp32 result, invisible
    next to the reassociation noise of any fp32 reference evaluation.
    So only t < 384 is loaded: 6 MB per batch instead of 16 MB.
  - layout: t = 3p + c -> SBUF tile [128 partitions, 3*4096]; each
    partition reads 48KB contiguous from HBM (single 6 MB DMA per batch
    measures ~400 GB/s/core).
  - w[3p+c] = w3[p] * mu^c with mu = e^(-1/tau): ACT scales column
    blocks c=1,2 by mu^c in place, DVE adds blocks into block 0, then
    one fp32 matmul per 512 columns with lhsT = w3 reduces the
    partition axis into PSUM; ACT copies PSUM->SBUF; DMA out.
"""

import numpy as np

import concourse.bacc as bacc
import concourse.bass as bass
import concourse.mybir as mybir
from concourse.bass_utils import run_bass_kernel_spmd
from concourse.tile import TileContext

B, T, N = 64, 1000, 4096
NCORES = 8
BL = B // NCORES  # batches per core
NCHUNK = 3  # t-blocks per partition; keeps t < 384 (see module docstring)
ROWS = 128 * NCHUNK  # 384 t-rows loaded per batch
TAU = 20.0
MM_N = 512  # fp32 matmul free-dim max (one PSUM bank)
NQ = 4  # n-slices for software pipelining


def _build(loop_iters: int = 0) -> bass.Bass:
    """Build the per-core program. loop_iters>1 wraps the whole program in
    a hardware For_i loop (benchmarking only)."""
    import contextlib

    nc = bacc.Bacc("TRN2", target_bir_lowering=False, debug=False)
    x = nc.dram_tensor("x", [BL, T, N], mybir.dt.float32, kind="ExternalInput")
    w = nc.dram_tensor("w", [128, 1], mybir.dt.float32, kind="ExternalInput")
    out = nc.dram_tensor("out", [BL, N], mybir.dt.float32, kind="ExternalOutput")
    mu = float(np.exp(-1.0 / TAU))
    NW = N // NQ  # n-slice width

    with TileContext(nc) as tc:
        with (
            tc.tile_pool(name="io", bufs=3) as io_pool,
            tc.tile_pool(name="wp", bufs=1) as w_pool,
            tc.tile_pool(name="op", bufs=2) as out_pool,
            tc.tile_pool(name="ps", bufs=4, space="PSUM") as psum_pool,
        ):
            w_tile = w_pool.tile([128, 1], mybir.dt.float32)
            nc.sync.dma_start(out=w_tile, in_=w[:, :])
            cm = tc.For_i(0, loop_iters, 1) if loop_iters > 1 else contextlib.nullcontext()
            with cm:
                for b in range(BL):
                    xt = io_pool.tile([128, NCHUNK * N], mybir.dt.float32, tag="xt")
                    src = x[b, 0:ROWS, :].rearrange("(p c) n -> p (c n)", p=128)
                    nc.sync.dma_start(out=xt, in_=src)
                    orow = out_pool.tile([1, N], mybir.dt.float32, tag="orow")
                    for q in range(NQ):
                        # scale blocks c>=1 by mu^c (ACT, in place)
                        for c in range(1, NCHUNK):
                            s_c = slice(c * N + q * NW, c * N + (q + 1) * NW)
                            nc.scalar.mul(xt[:, s_c], xt[:, s_c], mu**c)
                        # tree-add blocks into block 0 (DVE)
                        srcs = list(range(NCHUNK))
                        while len(srcs) > 1:
                            nxt = []
                            for k in range(0, len(srcs) - 1, 2):
                                a, bb = srcs[k], srcs[k + 1]
                                sa = slice(a * N + q * NW, a * N + (q + 1) * NW)
                                sb = slice(bb * N + q * NW, bb * N + (q + 1) * NW)
                                nc.vector.tensor_add(
                                    out=xt[:, sa], in0=xt[:, sa], in1=xt[:, sb]
                                )
                                nxt.append(a)
                            if len(srcs) % 2:
                                nxt.append(srcs[-1])
                            srcs = nxt
                        # partition-axis reduction with the weight column
                        ps_q = psum_pool.tile([1, NW], mybir.dt.float32, tag="ps")
                        for j in range(NW // MM_N):
                            nc.tensor.matmul(
                                ps_q[:, j * MM_N : (j + 1) * MM_N],
                                lhsT=w_tile[:, :],
                                rhs=xt[:, q * NW + j * MM_N : q * NW + (j + 1) * MM_N],
                                start=True,
                                stop=True,
                            )
                        nc.scalar.copy(orow[:, q * NW : (q + 1) * NW], ps_q[:, :])
                    nc.sync.dma_start(out=out[b : b + 1, :], in_=orow[:, :])
    nc.compile()
    return nc


_NC = None


def _get_nc() -> bass.Bass:
    global _NC
    if _NC is None:
        _NC = _build()
    return _NC


def _weights() -> np.ndarray:
    # replicate the reference weight computation in fp32, then take the
    # per-partition factor w3[p] = w[3p] (t = 3p + c decomposition)
    w = np.exp(-np.arange(T, dtype=np.float32) / np.float32(TAU))
    w = w / w.sum(dtype=np.float32)
    return np.ascontiguousarray(w[0:ROWS:NCHUNK].reshape(128, 1))


def kernel(spike_trains: np.ndarray, _trace: bool = False):
    assert spike_trains.shape == (B, T, N), spike_trains.shape
    x = np.ascontiguousarray(spike_trains, dtype=np.float32)
    w = _weights()
    in_maps = [
        {"x": np.ascontiguousarray(x[i * BL : (i + 1) * BL]), "w": w}
        for i in range(NCORES)
    ]
    res = run_bass_kernel_spmd(
        _get_nc(), in_maps, core_ids=list(range(NCORES)), trace=_trace
    )
    out = np.concatenate([r["out"] for r in res.results], axis=0)
    if _trace:
        return out, res
    return out


# revision 10
# speedup vs baseline: 108606.5850x; 108606.5850x over previous
"""Exponential smoother: out[b,n] = sum_t w[t] * x[b,t,n], with
w = normalized exp(-t/tau) decay weights (tau=20).

Strategy (8 NeuronCores, pure data parallel over B=64):
  - each core handles 8 batches of x[8, 1000, 4096] f32.
  - w decays so fast that t >= 384 contributes < 2.2e-9 absolute
    (~5e-9 relative) -- below half-ulp of the fp32 result, invisible
    next to the reassociation noise of any fp32 reference evaluation.
    So only t < 384 is loaded: 6 MB per batch instead of 16 MB.
  - layout: t = 3p + c -> SBUF tile [128 partitions, 3*4096]; each
    partition reads 48KB contiguous from HBM (single 6 MB DMA per batch
    measures ~400 GB/s/core).
  - w[3p+c] = w3[p] * mu^c with mu = e^(-1/tau): ACT scales column
    blocks c=1,2 by mu^c in place, DVE adds blocks into block 0, then
    one fp32 matmul per 512 columns with lhsT = w3 reduces the
    partition axis into PSUM; ACT copies PSUM->SBUF; DMA out.
"""

import numpy as np

import concourse.bacc as bacc
import concourse.bass as bass
import concourse.mybir as mybir
from concourse.bass_utils import run_bass_kernel_spmd
from concourse.tile import TileContext

B, T, N = 64, 1000, 4096
NCORES = 8
BL = B // NCORES  # batches per core
NCHUNK = 3  # t-blocks per partition; keeps t < 384 (see module docstring)
ROWS = 128 * NCHUNK  # 384 t-rows loaded per batch
TAU = 20.0
MM_N = 512  # fp32 matmul free-dim max (one PSUM bank)
NQ = 4  # n-slices for software pipelining


def _build(
    loop_iters: int = 0,
    nq: int = NQ,
    copy_eng: str = "scalar",
    diag: str | None = None,
    scale_split: bool = False,
) -> bass.Bass:
    """Build the per-core program. loop_iters>1 wraps the whole program in
    a hardware For_i loop; nq/copy_eng/diag/scale_split are benchmarking
    knobs (defaults = production)."""
    import contextlib

    nc = bacc.Bacc("TRN2", target_bir_lowering=False, debug=False)
    x = nc.dram_tensor("x", [BL, T, N], mybir.dt.float32, kind="ExternalInput")
    w = nc.dram_tensor("w", [128, 1], mybir.dt.float32, kind="ExternalInput")
    out = nc.dram_tensor("out", [BL, N], mybir.dt.float32, kind="ExternalOutput")
    mu = float(np.exp(-1.0 / TAU))
    NW = N // nq  # n-slice width

    with TileContext(nc) as tc:
        with (
            tc.tile_pool(name="io", bufs=3) as io_pool,
            tc.tile_pool(name="wp", bufs=1) as w_pool,
            tc.tile_pool(name="op", bufs=2) as out_pool,
            tc.tile_pool(name="ps", bufs=4, space="PSUM") as psum_pool,
        ):
            w_tile = w_pool.tile([128, 1], mybir.dt.float32)
            nc.sync.dma_start(out=w_tile, in_=w[:, :])
            cm = tc.For_i(0, loop_iters, 1) if loop_iters > 1 else contextlib.nullcontext()
            with cm:
                for b in range(BL):
                    xt = io_pool.tile([128, NCHUNK * N], mybir.dt.float32, tag="xt")
                    src = x[b, 0:ROWS, :].rearrange("(p c) n -> p (c n)", p=128)
                    nc.sync.dma_start(out=xt, in_=src)
                    orow = out_pool.tile([1, N], mybir.dt.float32, tag="orow")
                    for q in range(nq):
                        if diag != "noelem":
                            # scale blocks c>=1 by mu^c (in place)
                            for c in range(1, NCHUNK):
                                s_c = slice(c * N + q * NW, c * N + (q + 1) * NW)
                                if scale_split and c == 2:
                                    nc.vector.tensor_scalar_mul(
                                        xt[:, s_c], xt[:, s_c], mu**c
                                    )
                                else:
                                    nc.scalar.mul(xt[:, s_c], xt[:, s_c], mu**c)
                            # tree-add blocks into block 0 (DVE)
                            srcs = list(range(NCHUNK))
                            while len(srcs) > 1:
                                nxt = []
                                for k in range(0, len(srcs) - 1, 2):
                                    a, bb = srcs[k], srcs[k + 1]
                                    sa = slice(a * N + q * NW, a * N + (q + 1) * NW)
                                    sb = slice(bb * N + q * NW, bb * N + (q + 1) * NW)
                                    nc.vector.tensor_add(
                                        out=xt[:, sa], in0=xt[:, sa], in1=xt[:, sb]
                                    )
                                    nxt.append(a)
                                if len(srcs) % 2:
                                    nxt.append(srcs[-1])
                                srcs = nxt
                        ps_q = psum_pool.tile([1, NW], mybir.dt.float32, tag="ps")
                        if diag == "nomm":
                            nc.vector.tensor_copy(
                                out=ps_q[:, 0:8], in_=xt[0:1, q * NW : q * NW + 8]
                            )
                        else:
                            # partition-axis reduction with the weight column
                            for j in range(NW // MM_N):
                                nc.tensor.matmul(
                                    ps_q[:, j * MM_N : (j + 1) * MM_N],
                                    lhsT=w_tile[:, :],
                                    rhs=xt[
                                        :, q * NW + j * MM_N : q * NW + (j + 1) * MM_N
                                    ],
                                    start=True,
                                    stop=True,
                                )
                        if copy_eng == "scalar":
                            nc.scalar.copy(orow[:, q * NW : (q + 1) * NW], ps_q[:, :])
                        else:
                            nc.vector.tensor_copy(
                                out=orow[:, q * NW : (q + 1) * NW], in_=ps_q[:, :]
                            )
                    nc.sync.dma_start(out=out[b : b + 1, :], in_=orow[:, :])
    nc.compile()
    return nc


_NC = None


def _get_nc() -> bass.Bass:
    global _NC
    if _NC is None:
        _NC = _build()
    return _NC


def _weights() -> np.ndarray:
    # replicate the reference weight computation in fp32, then take the
    # per-partition factor w3[p] = w[3p] (t = 3p + c decomposition)
    w = np.exp(-np.arange(T, dtype=np.float32) / np.float32(TAU))
    w = w / w.sum(dtype=np.float32)
    return np.ascontiguousarray(w[0:ROWS:NCHUNK].reshape(128, 1))


def kernel(spike_trains: np.ndarray, _trace: bool = False):
    assert spike_trains.shape == (B, T, N), spike_trains.shape
    x = np.ascontiguousarray(spike_trains, dtype=np.float32)
    w = _weights()
    in_maps = [
        {"x": np.ascontiguousarray(x[i * BL : (i + 1) * BL]), "w": w}
        for i in range(NCORES)
    ]
    res = run_bass_kernel_spmd(
        _get_nc(), in_maps, core_ids=list(range(NCORES)), trace=_trace
    )
    out = np.concatenate([r["out"] for r in res.results], axis=0)
    if _trace:
        return out, res
    return out


# revision 14
# speedup vs baseline: 108929.6068x; 1.0030x over previous
"""Exponential smoother: out[b,n] = sum_t w[t] * x[b,t,n], with
w = normalized exp(-t/tau) decay weights (tau=20).

Strategy (8 NeuronCores, pure data parallel over B=64):
  - each core handles 8 batches of x[8, 1000, 4096] f32.
  - w decays so fast that t >= 384 contributes < 2.2e-9 absolute
    (~5e-9 relative) -- below half-ulp of the fp32 result, invisible
    next to the reassociation noise of any fp32 reference evaluation.
    So only t < 384 is loaded: 6 MB per batch instead of 16 MB.
  - layout: t = 3p + c -> SBUF tile [128 partitions, 3*4096]; each
    partition reads 48KB contiguous from HBM (single 6 MB DMA per batch
    measures ~400 GB/s/core).
  - w[3p+c] = w3[p] * mu^c with mu = e^(-1/tau): ACT scales column
    blocks c=1,2 by mu^c in place, DVE adds blocks into block 0, then
    one fp32 matmul per 512 columns with lhsT = w3 reduces the
    partition axis into PSUM; ACT copies PSUM->SBUF; DMA out.
"""

import numpy as np

import concourse.bacc as bacc
import concourse.bass as bass
import concourse.mybir as mybir
from concourse.bass_utils import run_bass_kernel_spmd
from concourse.tile import TileContext

B, T, N = 64, 1000, 4096
NCORES = 8
BL = B // NCORES  # batches per core
NCHUNK = 3  # t-blocks per partition; keeps t < 384 (see module docstring)
ROWS = 128 * NCHUNK  # 384 t-rows loaded per batch
TAU = 20.0
MM_N = 512  # fp32 matmul free-dim max (one PSUM bank)
NQ = 4  # n-slices for software pipelining


def _build(
    loop_iters: int = 0,
    nq: int = NQ,
    copy_eng: str = "scalar",
    diag: str | None = None,
    scale_split: bool = False,
) -> bass.Bass:
    """Build the per-core program. loop_iters>1 wraps the whole program in
    a hardware For_i loop; nq/copy_eng/diag/scale_split are benchmarking
    knobs (defaults = production)."""
    import contextlib

    nc = bacc.Bacc("TRN2", target_bir_lowering=False, debug=False)
    x = nc.dram_tensor("x", [BL, T, N], mybir.dt.float32, kind="ExternalInput")
    w = nc.dram_tensor("w", [128, 1], mybir.dt.float32, kind="ExternalInput")
    out = nc.dram_tensor("out", [BL, N], mybir.dt.float32, kind="ExternalOutput")
    mu = float(np.exp(-1.0 / TAU))
    NW = N // nq  # n-slice width

    with TileContext(nc) as tc:
        with (
            tc.tile_pool(name="io", bufs=3) as io_pool,
            tc.tile_pool(name="wp", bufs=1) as w_pool,
            tc.tile_pool(name="op", bufs=2) as out_pool,
            tc.tile_pool(name="ps", bufs=4, space="PSUM") as psum_pool,
        ):
            w_tile = w_pool.tile([128, 1], mybir.dt.float32)
            nc.sync.dma_start(out=w_tile, in_=w[:, :])
            cm = tc.For_i(0, loop_iters, 1) if loop_iters > 1 else contextlib.nullcontext()
            with cm:
                for b in range(BL):
                    xt = io_pool.tile([128, NCHUNK * N], mybir.dt.float32, tag="xt")
                    src = x[b, 0:ROWS, :].rearrange("(p c) n -> p (c n)", p=128)
                    nc.sync.dma_start(out=xt, in_=src)
                    orow = out_pool.tile([1, N], mybir.dt.float32, tag="orow")
                    for q in range(nq):
                        if diag != "noelem":
                            # scale blocks c>=1 by mu^c (in place)
                            for c in range(1, NCHUNK):
                                s_c = slice(c * N + q * NW, c * N + (q + 1) * NW)
                                if scale_split and c == 2:
                                    nc.vector.tensor_scalar_mul(
                                        xt[:, s_c], xt[:, s_c], mu**c
                                    )
                                else:
                                    nc.scalar.mul(xt[:, s_c], xt[:, s_c], mu**c)
                            # tree-add blocks into block 0 (DVE)
                            srcs = list(range(NCHUNK))
                            while len(srcs) > 1:
                                nxt = []
                                for k in range(0, len(srcs) - 1, 2):
                                    a, bb = srcs[k], srcs[k + 1]
                                    sa = slice(a * N + q * NW, a * N + (q + 1) * NW)
                                    sb = slice(bb * N + q * NW, bb * N + (q + 1) * NW)
                                    nc.vector.tensor_add(
                                        out=xt[:, sa], in0=xt[:, sa], in1=xt[:, sb]
                                    )
                                    nxt.append(a)
                                if len(srcs) % 2:
                                    nxt.append(srcs[-1])
                                srcs = nxt
                        ps_q = psum_pool.tile([1, NW], mybir.dt.float32, tag="ps")
                        if diag == "nomm":
                            nc.vector.tensor_copy(
                                out=ps_q[:, 0:8], in_=xt[0:1, q * NW : q * NW + 8]
                            )
                        else:
                            # partition-axis reduction with the weight column
                            for j in range(NW // MM_N):
                                nc.tensor.matmul(
                                    ps_q[:, j * MM_N : (j + 1) * MM_N],
                                    lhsT=w_tile[:, :],
                                    rhs=xt[
                                        :, q * NW + j * MM_N : q * NW + (j + 1) * MM_N
                                    ],
                                    start=True,
                                    stop=True,
                                )
                        if copy_eng == "scalar":
                            nc.scalar.copy(orow[:, q * NW : (q + 1) * NW], ps_q[:, :])
                        else:
                            nc.vector.tensor_copy(
                                out=orow[:, q * NW : (q + 1) * NW], in_=ps_q[:, :]
                            )
                    nc.sync.dma_start(out=out[b : b + 1, :], in_=orow[:, :])
    nc.compile()
    return nc


def _build2(
    loop_iters: int = 0,
    nq: int = 4,
    tail: bool = True,
    dma_only: bool = False,
) -> bass.Bass:
    """t = 2p + c main tile (t < 256) + optional 64-row tail tile
    (t in [256, 320)) folded in via per-partition ratio scale.
    5 MB per batch instead of 6 MB."""
    import contextlib

    nc = bacc.Bacc("TRN2", target_bir_lowering=False, debug=False)
    x = nc.dram_tensor("x", [BL, T, N], mybir.dt.float32, kind="ExternalInput")
    w = nc.dram_tensor("w", [128, 1], mybir.dt.float32, kind="ExternalInput")
    r = nc.dram_tensor("r", [64, 1], mybir.dt.float32, kind="ExternalInput")
    out = nc.dram_tensor("out", [BL, N], mybir.dt.float32, kind="ExternalOutput")
    mu = float(np.exp(-1.0 / TAU))
    NW = N // nq

    with TileContext(nc) as tc:
        with (
            tc.tile_pool(name="io", bufs=3) as io_pool,
            tc.tile_pool(name="tl", bufs=3) as tail_pool,
            tc.tile_pool(name="wp", bufs=1) as w_pool,
            tc.tile_pool(name="op", bufs=2) as out_pool,
            tc.tile_pool(name="ps", bufs=4, space="PSUM") as psum_pool,
        ):
            w_tile = w_pool.tile([128, 1], mybir.dt.float32)
            nc.sync.dma_start(out=w_tile, in_=w[:, :])
            r_tile = w_pool.tile([64, 1], mybir.dt.float32)
            nc.sync.dma_start(out=r_tile, in_=r[:, :])
            cm = (
                tc.For_i(0, loop_iters, 1)
                if loop_iters > 1
                else contextlib.nullcontext()
            )
            with cm:
                for b in range(BL):
                    xt = io_pool.tile([128, 2 * N], mybir.dt.float32, tag="xt")
                    nc.sync.dma_start(
                        out=xt,
                        in_=x[b, 0:256, :].rearrange("(p c) n -> p (c n)", p=128),
                    )
                    if tail:
                        xtl = tail_pool.tile([64, N], mybir.dt.float32, tag="xtl")
                        # scalar-engine HWDGE ring: keeps the 1MB tail DMA out
                        # of the SP ring carrying the 4MB main stream
                        nc.scalar.dma_start(out=xtl, in_=x[b, 256:320, :])
                    orow = out_pool.tile([1, N], mybir.dt.float32, tag="orow")
                    for q in range(nq):
                        sq = slice(q * NW, (q + 1) * NW)
                        s1 = slice(N + q * NW, N + (q + 1) * NW)
                        if not dma_only:
                            nc.scalar.mul(xt[:, s1], xt[:, s1], mu)
                            if tail:
                                nc.vector.tensor_scalar_mul(
                                    xtl[:, sq], xtl[:, sq], r_tile[:, :]
                                )
                            nc.vector.tensor_add(
                                out=xt[:, sq], in0=xt[:, sq], in1=xt[:, s1]
                            )
                            if tail:
                                nc.vector.tensor_add(
                                    out=xt[0:64, sq],
                                    in0=xt[0:64, sq],
                                    in1=xtl[:, sq],
                                )
                        ps_q = psum_pool.tile([1, NW], mybir.dt.float32, tag="ps")
                        if dma_only:
                            nc.vector.tensor_copy(
                                out=ps_q[:, 0:8], in_=xt[0:1, q * NW : q * NW + 8]
                            )
                        else:
                            for j in range(NW // MM_N):
                                nc.tensor.matmul(
                                    ps_q[:, j * MM_N : (j + 1) * MM_N],
                                    lhsT=w_tile[:, :],
                                    rhs=xt[:, q * NW + j * MM_N : q * NW + (j + 1) * MM_N],
                                    start=True,
                                    stop=True,
                                )
                        nc.scalar.copy(orow[:, sq], ps_q[:, :])
                    nc.sync.dma_start(out=out[b : b + 1, :], in_=orow[:, :])
    nc.compile()
    return nc


def _build3(
    loop_iters: int = 0,
    nq: int = 4,
    dma_only: bool = False,
) -> bass.Bass:
    """t = 2p + c (t < 256), TWO batches per DMA (8 MB) to stay in the
    big-transfer DMA-efficiency regime. Tile [128, 2*2*N]: cols =
    (b2, c, n); per partition two contiguous 32KB source chunks."""
    import contextlib

    nc = bacc.Bacc("TRN2", target_bir_lowering=False, debug=False)
    x = nc.dram_tensor("x", [BL, T, N], mybir.dt.float32, kind="ExternalInput")
    w = nc.dram_tensor("w", [128, 1], mybir.dt.float32, kind="ExternalInput")
    r = nc.dram_tensor("r", [64, 1], mybir.dt.float32, kind="ExternalInput")
    out = nc.dram_tensor("out", [BL, N], mybir.dt.float32, kind="ExternalOutput")
    mu = float(np.exp(-1.0 / TAU))
    NW = N // nq

    with TileContext(nc) as tc:
        with (
            tc.tile_pool(name="io", bufs=2) as io_pool,
            tc.tile_pool(name="wp", bufs=1) as w_pool,
            tc.tile_pool(name="op", bufs=2) as out_pool,
            tc.tile_pool(name="ps", bufs=4, space="PSUM") as psum_pool,
        ):
            w_tile = w_pool.tile([128, 1], mybir.dt.float32)
            nc.sync.dma_start(out=w_tile, in_=w[:, :])
            cm = (
                tc.For_i(0, loop_iters, 1)
                if loop_iters > 1
                else contextlib.nullcontext()
            )
            with cm:
                for bp in range(BL // 2):
                    xt = io_pool.tile([128, 2, 2, N], mybir.dt.float32, tag="xt")
                    src = x[2 * bp : 2 * bp + 2, 0:256, :].rearrange(
                        "b (p c) n -> p b c n", p=128
                    )
                    nc.sync.dma_start(out=xt, in_=src)
                    for b2 in range(2):
                        b = 2 * bp + b2
                        orow = out_pool.tile([1, N], mybir.dt.float32, tag="orow")
                        for q in range(nq):
                            sq = slice(q * NW, (q + 1) * NW)
                            if not dma_only:
                                nc.scalar.mul(
                                    xt[:, b2, 1, sq], xt[:, b2, 1, sq], mu
                                )
                                nc.vector.tensor_add(
                                    out=xt[:, b2, 0, sq],
                                    in0=xt[:, b2, 0, sq],
                                    in1=xt[:, b2, 1, sq],
                                )
                            ps_q = psum_pool.tile([1, NW], mybir.dt.float32, tag="ps")
                            if dma_only:
                                nc.vector.tensor_copy(
                                    out=ps_q[:, 0:8], in_=xt[0:1, b2, 0, 0:8]
                                )
                            else:
                                for j in range(NW // MM_N):
                                    nc.tensor.matmul(
                                        ps_q[:, j * MM_N : (j + 1) * MM_N],
                                        lhsT=w_tile[:, :],
                                        rhs=xt[
                                            :,
                                            b2,
                                            0,
                                            q * NW + j * MM_N : q * NW
                                            + (j + 1) * MM_N,
                                        ],
                                        start=True,
                                        stop=True,
                                    )
                            nc.scalar.copy(
                                orow[:, q * NW : (q + 1) * NW], ps_q[:, :]
                            )
                        nc.sync.dma_start(out=out[b : b + 1, :], in_=orow[:, :])
    nc.compile()
    return nc


def _weights2():
    w = np.exp(-np.arange(T, dtype=np.float32) / np.float32(TAU))
    w = w / w.sum(dtype=np.float32)
    w2 = np.ascontiguousarray(w[0:256:2].reshape(128, 1))
    r = np.ascontiguousarray((w[256:320] / w[0:128:2][:64]).reshape(64, 1))
    return w2, r


_NC = None


def _get_nc() -> bass.Bass:
    global _NC
    if _NC is None:
        _NC = _build()
    return _NC


def _weights() -> np.ndarray:
    # replicate the reference weight computation in fp32, then take the
    # per-partition factor w3[p] = w[3p] (t = 3p + c decomposition)
    w = np.exp(-np.arange(T, dtype=np.float32) / np.float32(TAU))
    w = w / w.sum(dtype=np.float32)
    return np.ascontiguousarray(w[0:ROWS:NCHUNK].reshape(128, 1))


def kernel(spike_trains: np.ndarray, _trace: bool = False):
    assert spike_trains.shape == (B, T, N), spike_trains.shape
    x = np.ascontiguousarray(spike_trains, dtype=np.float32)
    w = _weights()
    in_maps = [
        {"x": np.ascontiguousarray(x[i * BL : (i + 1) * BL]), "w": w}
        for i in range(NCORES)
    ]
    res = run_bass_kernel_spmd(
        _get_nc(), in_maps, core_ids=list(range(NCORES)), trace=_trace
    )
    out = np.concatenate([r["out"] for r in res.results], axis=0)
    if _trace:
        return out, res
    return out


# revision 18
# speedup vs baseline: 117999.8512x; 1.0833x over previous
"""Exponential smoother: out[b,n] = sum_t w[t] * x[b,t,n], with
w = normalized exp(-t/tau) decay weights (tau=20).

Strategy (8 NeuronCores, pure data parallel over B=64):
  - each core handles 8 batches of x[8, 1000, 4096] f32.
  - w decays so fast that t >= 384 contributes < 2.2e-9 absolute
    (~5e-9 relative) -- below half-ulp of the fp32 result, invisible
    next to the reassociation noise of any fp32 reference evaluation.
    So only t < 384 is loaded: 6 MB per batch instead of 16 MB.
  - layout: t = 3p + c -> SBUF tile [128 partitions, 3*4096]; each
    partition reads 48KB contiguous from HBM (single 6 MB DMA per batch
    measures ~400 GB/s/core).
  - w[3p+c] = w3[p] * mu^c with mu = e^(-1/tau): ACT scales column
    blocks c=1,2 by mu^c in place, DVE adds blocks into block 0, then
    one fp32 matmul per 512 columns with lhsT = w3 reduces the
    partition axis into PSUM; ACT copies PSUM->SBUF; DMA out.
"""

import numpy as np

import concourse.bacc as bacc
import concourse.bass as bass
import concourse.mybir as mybir
from concourse.bass_utils import run_bass_kernel_spmd
from concourse.tile import TileContext

B, T, N = 64, 1000, 4096
NCORES = 8
BL = B // NCORES  # batches per core
NCHUNK = 3  # t-blocks per partition; keeps t < 384 (see module docstring)
ROWS = 128 * NCHUNK  # 384 t-rows loaded per batch
TAU = 20.0
MM_N = 512  # fp32 matmul free-dim max (one PSUM bank)
NQ = 4  # n-slices for software pipelining


def _build(
    loop_iters: int = 0,
    nq: int = NQ,
    copy_eng: str = "scalar",
    diag: str | None = None,
    scale_split: bool = False,
    split_ends: bool = False,
    out_ring: str = "sync",
) -> bass.Bass:
    """Build the per-core program. loop_iters>1 wraps the whole program in
    a hardware For_i loop; nq/copy_eng/diag/scale_split are benchmarking
    knobs (defaults = production)."""
    import contextlib

    nc = bacc.Bacc("TRN2", target_bir_lowering=False, debug=False)
    x = nc.dram_tensor("x", [BL, T, N], mybir.dt.float32, kind="ExternalInput")
    w = nc.dram_tensor("w", [128, 1], mybir.dt.float32, kind="ExternalInput")
    out = nc.dram_tensor("out", [BL, N], mybir.dt.float32, kind="ExternalOutput")
    mu = float(np.exp(-1.0 / TAU))
    NW = N // nq  # n-slice width

    with TileContext(nc) as tc:
        with (
            tc.tile_pool(name="io", bufs=3) as io_pool,
            tc.tile_pool(name="wp", bufs=1) as w_pool,
            tc.tile_pool(name="op", bufs=2) as out_pool,
            tc.tile_pool(name="ps", bufs=4, space="PSUM") as psum_pool,
        ):
            w_tile = w_pool.tile([128, 1], mybir.dt.float32)
            nc.sync.dma_start(out=w_tile, in_=w[:, :])
            cm = tc.For_i(0, loop_iters, 1) if loop_iters > 1 else contextlib.nullcontext()
            with cm:
                for b in range(BL):
                    xt = io_pool.tile([128, NCHUNK * N], mybir.dt.float32, tag="xt")
                    src = x[b, 0:ROWS, :].rearrange("(p c) n -> p (c n)", p=128)
                    if split_ends and b in (0, BL - 1):
                        # fill/drain trim: n-half split aligned with q deps
                        xt3 = xt.rearrange("p (c n) -> p c n", c=NCHUNK)
                        src3 = x[b, 0:ROWS, :].rearrange("(p c) n -> p c n", p=128)
                        h = N // 2
                        nc.sync.dma_start(out=xt3[:, :, 0:h], in_=src3[:, :, 0:h])
                        nc.sync.dma_start(out=xt3[:, :, h:N], in_=src3[:, :, h:N])
                    else:
                        nc.sync.dma_start(out=xt, in_=src)
                    orow = out_pool.tile([1, N], mybir.dt.float32, tag="orow")
                    for q in range(nq):
                        if diag != "noelem":
                            # scale blocks c>=1 by mu^c (in place)
                            for c in range(1, NCHUNK):
                                s_c = slice(c * N + q * NW, c * N + (q + 1) * NW)
                                if scale_split and c == 2:
                                    nc.vector.tensor_scalar_mul(
                                        xt[:, s_c], xt[:, s_c], mu**c
                                    )
                                else:
                                    nc.scalar.mul(xt[:, s_c], xt[:, s_c], mu**c)
                            # tree-add blocks into block 0 (DVE)
                            srcs = list(range(NCHUNK))
                            while len(srcs) > 1:
                                nxt = []
                                for k in range(0, len(srcs) - 1, 2):
                                    a, bb = srcs[k], srcs[k + 1]
                                    sa = slice(a * N + q * NW, a * N + (q + 1) * NW)
                                    sb = slice(bb * N + q * NW, bb * N + (q + 1) * NW)
                                    nc.vector.tensor_add(
                                        out=xt[:, sa], in0=xt[:, sa], in1=xt[:, sb]
                                    )
                                    nxt.append(a)
                                if len(srcs) % 2:
                                    nxt.append(srcs[-1])
                                srcs = nxt
                        ps_q = psum_pool.tile([1, NW], mybir.dt.float32, tag="ps")
                        if diag == "nomm":
                            nc.vector.tensor_copy(
                                out=ps_q[:, 0:8], in_=xt[0:1, q * NW : q * NW + 8]
                            )
                        else:
                            # partition-axis reduction with the weight column
                            for j in range(NW // MM_N):
                                nc.tensor.matmul(
                                    ps_q[:, j * MM_N : (j + 1) * MM_N],
                                    lhsT=w_tile[:, :],
                                    rhs=xt[
                                        :, q * NW + j * MM_N : q * NW + (j + 1) * MM_N
                                    ],
                                    start=True,
                                    stop=True,
                                )
                        if copy_eng == "scalar":
                            nc.scalar.copy(orow[:, q * NW : (q + 1) * NW], ps_q[:, :])
                        else:
                            nc.vector.tensor_copy(
                                out=orow[:, q * NW : (q + 1) * NW], in_=ps_q[:, :]
                            )
                    out_dma = nc.sync if out_ring == "sync" else nc.scalar
                    out_dma.dma_start(out=out[b : b + 1, :], in_=orow[:, :])
    nc.compile()
    return nc


def _build2(
    loop_iters: int = 0,
    nq: int = 4,
    tail: bool = True,
    dma_only: bool = False,
) -> bass.Bass:
    """t = 2p + c main tile (t < 256) + optional 64-row tail tile
    (t in [256, 320)) folded in via per-partition ratio scale.
    5 MB per batch instead of 6 MB."""
    import contextlib

    nc = bacc.Bacc("TRN2", target_bir_lowering=False, debug=False)
    x = nc.dram_tensor("x", [BL, T, N], mybir.dt.float32, kind="ExternalInput")
    w = nc.dram_tensor("w", [128, 1], mybir.dt.float32, kind="ExternalInput")
    r = nc.dram_tensor("r", [64, 1], mybir.dt.float32, kind="ExternalInput")
    out = nc.dram_tensor("out", [BL, N], mybir.dt.float32, kind="ExternalOutput")
    mu = float(np.exp(-1.0 / TAU))
    NW = N // nq

    with TileContext(nc) as tc:
        with (
            tc.tile_pool(name="io", bufs=3) as io_pool,
            tc.tile_pool(name="tl", bufs=3) as tail_pool,
            tc.tile_pool(name="wp", bufs=1) as w_pool,
            tc.tile_pool(name="op", bufs=2) as out_pool,
            tc.tile_pool(name="ps", bufs=4, space="PSUM") as psum_pool,
        ):
            w_tile = w_pool.tile([128, 1], mybir.dt.float32)
            nc.sync.dma_start(out=w_tile, in_=w[:, :])
            r_tile = w_pool.tile([64, 1], mybir.dt.float32)
            nc.sync.dma_start(out=r_tile, in_=r[:, :])
            cm = (
                tc.For_i(0, loop_iters, 1)
                if loop_iters > 1
                else contextlib.nullcontext()
            )
            with cm:
                for b in range(BL):
                    xt = io_pool.tile([128, 2 * N], mybir.dt.float32, tag="xt")
                    nc.sync.dma_start(
                        out=xt,
                        in_=x[b, 0:256, :].rearrange("(p c) n -> p (c n)", p=128),
                    )
                    if tail:
                        xtl = tail_pool.tile([64, N], mybir.dt.float32, tag="xtl")
                        # scalar-engine HWDGE ring: keeps the 1MB tail DMA out
                        # of the SP ring carrying the 4MB main stream
                        nc.scalar.dma_start(out=xtl, in_=x[b, 256:320, :])
                    orow = out_pool.tile([1, N], mybir.dt.float32, tag="orow")
                    for q in range(nq):
                        sq = slice(q * NW, (q + 1) * NW)
                        s1 = slice(N + q * NW, N + (q + 1) * NW)
                        if not dma_only:
                            nc.scalar.mul(xt[:, s1], xt[:, s1], mu)
                            if tail:
                                nc.vector.tensor_scalar_mul(
                                    xtl[:, sq], xtl[:, sq], r_tile[:, :]
                                )
                            nc.vector.tensor_add(
                                out=xt[:, sq], in0=xt[:, sq], in1=xt[:, s1]
                            )
                            if tail:
                                nc.vector.tensor_add(
                                    out=xt[0:64, sq],
                                    in0=xt[0:64, sq],
                                    in1=xtl[:, sq],
                                )
                        ps_q = psum_pool.tile([1, NW], mybir.dt.float32, tag="ps")
                        if dma_only:
                            nc.vector.tensor_copy(
                                out=ps_q[:, 0:8], in_=xt[0:1, q * NW : q * NW + 8]
                            )
                        else:
                            for j in range(NW // MM_N):
                                nc.tensor.matmul(
                                    ps_q[:, j * MM_N : (j + 1) * MM_N],
                                    lhsT=w_tile[:, :],
                                    rhs=xt[:, q * NW + j * MM_N : q * NW + (j + 1) * MM_N],
                                    start=True,
                                    stop=True,
                                )
                        nc.scalar.copy(orow[:, sq], ps_q[:, :])
                    nc.sync.dma_start(out=out[b : b + 1, :], in_=orow[:, :])
    nc.compile()
    return nc


def _build3(
    loop_iters: int = 0,
    nq: int = 4,
    dma_only: bool = False,
) -> bass.Bass:
    """t = 2p + c (t < 256), TWO batches per DMA (8 MB) to stay in the
    big-transfer DMA-efficiency regime. Tile [128, 2*2*N]: cols =
    (b2, c, n); per partition two contiguous 32KB source chunks."""
    import contextlib

    nc = bacc.Bacc("TRN2", target_bir_lowering=False, debug=False)
    x = nc.dram_tensor("x", [BL, T, N], mybir.dt.float32, kind="ExternalInput")
    w = nc.dram_tensor("w", [128, 1], mybir.dt.float32, kind="ExternalInput")
    r = nc.dram_tensor("r", [64, 1], mybir.dt.float32, kind="ExternalInput")
    out = nc.dram_tensor("out", [BL, N], mybir.dt.float32, kind="ExternalOutput")
    mu = float(np.exp(-1.0 / TAU))
    NW = N // nq

    with TileContext(nc) as tc:
        with (
            tc.tile_pool(name="io", bufs=2) as io_pool,
            tc.tile_pool(name="wp", bufs=1) as w_pool,
            tc.tile_pool(name="op", bufs=2) as out_pool,
            tc.tile_pool(name="ps", bufs=4, space="PSUM") as psum_pool,
        ):
            w_tile = w_pool.tile([128, 1], mybir.dt.float32)
            nc.sync.dma_start(out=w_tile, in_=w[:, :])
            cm = (
                tc.For_i(0, loop_iters, 1)
                if loop_iters > 1
                else contextlib.nullcontext()
            )
            with cm:
                for bp in range(BL // 2):
                    xt = io_pool.tile([128, 2, 2, N], mybir.dt.float32, tag="xt")
                    src = x[2 * bp : 2 * bp + 2, 0:256, :].rearrange(
                        "b (p c) n -> p b c n", p=128
                    )
                    nc.sync.dma_start(out=xt, in_=src)
                    for b2 in range(2):
                        b = 2 * bp + b2
                        orow = out_pool.tile([1, N], mybir.dt.float32, tag="orow")
                        for q in range(nq):
                            sq = slice(q * NW, (q + 1) * NW)
                            if not dma_only:
                                nc.scalar.mul(
                                    xt[:, b2, 1, sq], xt[:, b2, 1, sq], mu
                                )
                                nc.vector.tensor_add(
                                    out=xt[:, b2, 0, sq],
                                    in0=xt[:, b2, 0, sq],
                                    in1=xt[:, b2, 1, sq],
                                )
                            ps_q = psum_pool.tile([1, NW], mybir.dt.float32, tag="ps")
                            if dma_only:
                                nc.vector.tensor_copy(
                                    out=ps_q[:, 0:8], in_=xt[0:1, b2, 0, 0:8]
                                )
                            else:
                                for j in range(NW // MM_N):
                                    nc.tensor.matmul(
                                        ps_q[:, j * MM_N : (j + 1) * MM_N],
                                        lhsT=w_tile[:, :],
                                        rhs=xt[
                                            :,
                                            b2,
                                            0,
                                            q * NW + j * MM_N : q * NW
                                            + (j + 1) * MM_N,
                                        ],
                                        start=True,
                                        stop=True,
                                    )
                            nc.scalar.copy(
                                orow[:, q * NW : (q + 1) * NW], ps_q[:, :]
                            )
                        nc.sync.dma_start(out=out[b : b + 1, :], in_=orow[:, :])
    nc.compile()
    return nc


def _weights2():
    w = np.exp(-np.arange(T, dtype=np.float32) / np.float32(TAU))
    w = w / w.sum(dtype=np.float32)
    w2 = np.ascontiguousarray(w[0:256:2].reshape(128, 1))
    r = np.ascontiguousarray((w[256:320] / w[0:128:2][:64]).reshape(64, 1))
    return w2, r


_NC = None


def _get_nc() -> bass.Bass:
    global _NC
    if _NC is None:
        _NC = _build()
    return _NC


def _weights() -> np.ndarray:
    # replicate the reference weight computation in fp32, then take the
    # per-partition factor w3[p] = w[3p] (t = 3p + c decomposition)
    w = np.exp(-np.arange(T, dtype=np.float32) / np.float32(TAU))
    w = w / w.sum(dtype=np.float32)
    return np.ascontiguousarray(w[0:ROWS:NCHUNK].reshape(128, 1))


def kernel(spike_trains: np.ndarray, _trace: bool = False):
    assert spike_trains.shape == (B, T, N), spike_trains.shape
    x = np.ascontiguousarray(spike_trains, dtype=np.float32)
    w = _weights()
    in_maps = [
        {"x": np.ascontiguousarray(x[i * BL : (i + 1) * BL]), "w": w}
        for i in range(NCORES)
    ]
    res = run_bass_kernel_spmd(
        _get_nc(), in_maps, core_ids=list(range(NCORES)), trace=_trace
    )
    out = np.concatenate([r["out"] for r in res.results], axis=0)
    if _trace:
        return out, res
    return out
